# revision 39
# baseline (speedup 1.0000x reference)
"""Trainium2 Bass kernel for causal self-attention with ALiBi + GQA.

Problem: B=2, T=2048, C=2048, 16 q-heads / 4 kv-heads, head_dim=128.
  q = x@q_w.T, k = x@k_w.T, v = x@v_w.T (GQA repeat 4x)
  att = softmax(q k^T/sqrt(d) + causal + alibi); out = (att v) @ o_w.T

Sharding over 8 NeuronCores: core c -> batch c//4, kv-group g=c%4
(q-heads 4g..4g+3, kv-head g).  Each core computes attention for its 4
heads on its batch plus a partial o-projection over its 512 channels;
the host sums the 4 partials per batch.

On-chip design v2 (per core, fp16 matmuls, fp32 accumulate):
  - Projections make QT [d,t], KT [d,t] and VT [d,t]; VT is PE-transposed
    back to V natural [t,d] blocks (V as AV stationary).
  - Scores transposed sT[k,q] = KT_blk.T @ QT (moving free dim 512,
    causally narrowed per diagonal offset).  Causal+ALiBi additive tiles
    (f32, DVE) added in-place in PSUM; for late-chunk small-slope head
    slots the ALiBi term is instead a rank-2 fp16 matmul accumulated into
    the same PSUM (balances DVE vs PE).  exp on ACT straight from PSUM
    with per-(head,offset) bias, fp16 out.
  - AV computed transposed directly: yT[d,q] += V_nat[kb].T @ et[kb]
    (et is the moving operand, 512-wide) -- no per-qblock small matmuls,
    no yT transposes.
  - Softmax denominators: et tiles accumulated on DVE (fp16, 4x mode,
    4 chains to bound rounding), cross-partition summed+broadcast by
    GPSIMD partition_all_reduce, reciprocal on DVE, and the normalize
    multiply is fused with the yT PSUM->fp16 cast.
  - o-projection and next-chunk projections are woven between attention
    tiles so the PE never drains while DVE/ACT chew the softmax chain.
"""

import math
import sys
from collections import deque
from contextlib import ExitStack

import numpy as np

sys.path.insert(0, "/opt/trn_rl_repo")

import ml_dtypes  # noqa: E402,F401

import concourse.bacc as bacc  # noqa: E402
import concourse.bass as bass  # noqa: E402,F401
import concourse.bass_isa as bass_isa  # noqa: E402
import concourse.mybir as mybir  # noqa: E402
import concourse.tile as tile  # noqa: E402

F16 = mybir.dt.float16
F32 = mybir.dt.float32
NP_F16 = np.float16

B, T, C = 2, 2048, 2048
H, HKV, D = 16, 4, 128
P = 128
CH = 512                 # q-chunk (moving free dim)
NCB = C // P             # 16 contraction blocks
NTB = T // P             # 16 t-blocks
NCHK = T // CH           # 4 q-chunks
NQH = 4                  # local q heads per core
SCALE = 1.0 / math.sqrt(D)
MASK_NEG = -1.0e9

# (j, slot) pairs whose non-diagonal tiles get the rank-2 PE alibi
# instead of the DVE mcl add (late chunks are DVE-bound otherwise).
def _use_rank2(j, slot, oi):
    return oi < 0 and j == 3 and slot >= 2


def _alibi_slopes(n):
    start = 2 ** (-(2 ** (-(math.log2(n) - 3))))
    return np.array([start * start**i for i in range(n)], dtype=np.float64)


def build_program():
    """Build the (SPMD-identical) single-core program."""
    nc = bacc.Bacc("TRN2", target_bir_lowering=False, debug=False, num_devices=8)

    xT_ap = nc.dram_tensor("xT", [P, NCB, T], F16, kind="ExternalInput").ap()
    qwT_ap = nc.dram_tensor("qwT", [P, NCB, NQH * P], F16, kind="ExternalInput").ap()
    kwT_ap = nc.dram_tensor("kwT", [P, NCB, D], F16, kind="ExternalInput").ap()
    vwT_ap = nc.dram_tensor("vwT", [P, NCB, D], F16, kind="ExternalInput").ap()
    owT_ap = nc.dram_tensor("owT", [P, NQH, C], F16, kind="ExternalInput").ap()
    mcl_ap = nc.dram_tensor("mcl", [P, NQH, CH], F32, kind="ExternalInput").ap()
    mcb_ap = nc.dram_tensor("mcb", [P, NQH, 4, CH], F32, kind="ExternalInput").ap()
    bias_ap = nc.dram_tensor("bias", [P, NQH, 32], F32, kind="ExternalInput").ap()
    r2l_ap = nc.dram_tensor("r2l", [2, NQH, P], F16, kind="ExternalInput").ap()
    r2r_ap = nc.dram_tensor("r2r", [2, NQH, CH], F16, kind="ExternalInput").ap()
    id_ap = nc.dram_tensor("ident", [P, P], F16, kind="ExternalInput").ap()
    ones_ap = nc.dram_tensor("ones", [P, 1], F16, kind="ExternalInput").ap()
    onesr_ap = nc.dram_tensor("onesr", [1, P], F16, kind="ExternalInput").ap()
    out_ap = nc.dram_tensor("out_p", [T, C], F16, kind="ExternalOutput").ap()

    EXP = mybir.ActivationFunctionType.Exp

    with tile.TileContext(nc) as tc, ExitStack() as ctx:
        const = ctx.enter_context(tc.tile_pool(name="const", bufs=1))
        qwT_sb = const.tile([P, NCB, NQH * P], F16, name="qwT_sb")
        kwT_sb = const.tile([P, NCB, D], F16, name="kwT_sb")
        vwT_sb = const.tile([P, NCB, D], F16, name="vwT_sb")
        owT_sb = const.tile([P, NQH, C], F16, name="owT_sb")
        mcl_sb = const.tile([P, NQH, CH], F32, name="mcl_sb")
        mcb_sb = const.tile([P, NQH, 4, CH], F32, name="mcb_sb")
        bias_sb = const.tile([P, NQH, 32], F32, name="bias_sb")
        r2l_sb = const.tile([2, NQH, P], F16, name="r2l_sb")
        r2r_sb = const.tile([2, NQH, CH], F16, name="r2r_sb")
        id_sb = const.tile([P, P], F16, name="id_sb")
        ones_sb = const.tile([P, 1], F16, name="ones_sb")
        onesr_sb = const.tile([1, P], F16, name="onesr_sb")

        QT_sb = const.tile([P, NQH, T], F16, name="QT_sb")
        KT_sb = const.tile([P, T], F16, name="KT_sb")
        Vn_sb = const.tile([P, NTB, D], F16, name="Vn_sb")
        yT_sb = const.tile([P, NQH, T], F16, name="yT_sb")

        xT_pool = ctx.enter_context(tc.tile_pool(name="xT_pool", bufs=9))
        sc_pool = ctx.enter_context(tc.tile_pool(name="sc_pool", bufs=3, space="PSUM"))
        yps_pool = ctx.enter_context(tc.tile_pool(name="yps_pool", bufs=2, space="PSUM"))
        wk_pool = ctx.enter_context(tc.tile_pool(name="wk_pool", bufs=3, space="PSUM"))
        et_pool = ctx.enter_context(tc.tile_pool(name="et_pool", bufs=19))
        vts_pool = ctx.enter_context(tc.tile_pool(name="vts_pool", bufs=2))
        rt_pool = ctx.enter_context(tc.tile_pool(name="rt_pool", bufs=2))
        row_pool = ctx.enter_context(tc.tile_pool(name="row_pool", bufs=2))
        rbc_pool = ctx.enter_context(tc.tile_pool(name="rbc_pool", bufs=2))
        oev_pool = ctx.enter_context(tc.tile_pool(name="oev_pool", bufs=6))

        # ---------- PE-filler weaving machinery ----------
        filler = deque()  # each entry: zero-arg fn emitting one psum-group

        def drain_filler(n):
            for _ in range(n):
                if not filler:
                    return
                filler.popleft()()

        def drain_all_filler():
            while filler:
                filler.popleft()()

        # ---------- projection units for chunk j ----------
        def dma_x_chunk(j):
            t0 = j * CH
            xts = []
            for q in range(4):
                xq = xT_pool.tile([P, 4, CH], F16, name=f"xq_{j}_{q}", tag="xq")
                nc.sync.dma_start(xq[:], xT_ap[:, 4 * q : 4 * q + 4, t0 : t0 + CH])
                for i in range(4):
                    xts.append(xq[:, i, :])
            return xts

        def unit_qproj(j, xts, qh):
            def emit():
                t0 = j * CH
                ps = wk_pool.tile([P, CH], F32, name=f"psq_{j}_{qh}", tag="wk")
                for cb in range(NCB):
                    nc.tensor.matmul(
                        ps[:],
                        lhsT=qwT_sb[:, cb, qh * P : (qh + 1) * P],
                        rhs=xts[cb][:],
                        start=(cb == 0),
                        stop=(cb == NCB - 1),
                    )
                nc.scalar.copy(QT_sb[:, qh, t0 : t0 + CH], ps[:])
            return emit

        def unit_kproj(j, xts):
            def emit():
                t0 = j * CH
                ps = wk_pool.tile([P, CH], F32, name=f"psk_{j}", tag="wk")
                for cb in range(NCB):
                    nc.tensor.matmul(
                        ps[:],
                        lhsT=kwT_sb[:, cb, :],
                        rhs=xts[cb][:],
                        start=(cb == 0),
                        stop=(cb == NCB - 1),
                    )
                nc.scalar.copy(KT_sb[:, t0 : t0 + CH], ps[:])
            return emit

        def unit_vproj(j, xts):
            def emit():
                ps = wk_pool.tile([P, CH], F32, name=f"psv_{j}", tag="wk")
                for cb in range(NCB):
                    nc.tensor.matmul(
                        ps[:],
                        lhsT=vwT_sb[:, cb, :],
                        rhs=xts[cb][:],
                        start=(cb == 0),
                        stop=(cb == NCB - 1),
                    )
                vts = vts_pool.tile([P, CH], F16, name=f"vts_{j}", tag="vts")
                nc.scalar.copy(vts[:], ps[:])
                tp = wk_pool.tile([P, CH], F16, name=f"vtp_{j}", tag="wk")
                for qb in range(CH // P):
                    nc.tensor.transpose(
                        tp[:, qb * P : (qb + 1) * P],
                        vts[:, qb * P : (qb + 1) * P],
                        id_sb[:],
                    )
                nc.vector.tensor_copy(Vn_sb[:, 4 * j : 4 * j + 4, :], tp[:])
            return emit

        # ---------- o-projection units for chunk j ----------
        def unit_oproj(tb, nch):
            def emit():
                pso = wk_pool.tile([P, CH], F32, name=f"pso_{tb}_{nch}", tag="wk")
                for hb in range(NQH):
                    nc.tensor.matmul(
                        pso[:],
                        lhsT=yT_sb[:, hb, tb * P : (tb + 1) * P],
                        rhs=owT_sb[:, hb, nch * CH : (nch + 1) * CH],
                        start=(hb == 0),
                        stop=(hb == NQH - 1),
                    )
                ot = oev_pool.tile([P, CH], F16, name=f"ot_{tb}_{nch}", tag="ot")
                nc.scalar.copy(ot[:], pso[:])
                nc.sync.dma_start(
                    out_ap[tb * P : (tb + 1) * P, nch * CH : (nch + 1) * CH], ot[:]
                )
            return emit

        # ---------- attention for chunk j (with weaving) ----------
        def attention_chunk(j):
            q0 = j * CH
            nkb = 4 * j + 4
            n_slots = NQH * nkb
            n_units0 = len(filler)
            emitted = [0]

            def weave(slot_idx):
                want = ((slot_idx + 1) * n_units0) // n_slots
                drain_filler(want - emitted[0])
                emitted[0] = want

            def denom_cluster(h, acc, yps):
                # all of head h's AV matmuls must be emitted before reading yps
                while pending_av and pending_av[0][0] == h:
                    pending_av.popleft()[1]()
                # column sums of acc -> dT [q,4] via 4 one-col matmuls
                dT = wk_pool.tile([P, 4], F32, name=f"dT_{h}_{j}", tag="wk")
                for qb in range(4):
                    nc.tensor.matmul(
                        dT[:, qb : qb + 1],
                        lhsT=acc[:, qb * P : (qb + 1) * P],
                        rhs=ones_sb[:],
                        start=True,
                        stop=True,
                    )
                rt = rt_pool.tile([P, 4], F16, name=f"rt_{h}_{j}", tag="rt")
                with nc.allow_low_precision("f16 softmax recip, 1e-3 ok at 2e-2 gate"):
                    nc.vector.reciprocal(rt[:], dT[:])
                rowp = wk_pool.tile([1, CH], F16, name=f"rowp_{h}_{j}", tag="wk")
                for qb in range(4):
                    nc.tensor.transpose(
                        rowp[:, qb * P : (qb + 1) * P], rt[:, qb : qb + 1], id_sb[:]
                    )
                rows = row_pool.tile([1, CH], F16, name=f"rows_{h}_{j}", tag="row")
                nc.vector.tensor_copy(rows[:], rowp[:])
                rbc = rbc_pool.tile([P, CH], F16, name=f"rbc_{h}_{j}", tag="rbc")
                nc.gpsimd.partition_broadcast(rbc[:], rows[:], channels=P)
                nc.vector.tensor_mul(yT_sb[:, h, q0 : q0 + CH], yps[:], rbc[:])

            pending_denom = [None]
            pending_av = deque()  # (head, av-thunk), emitted 5 tiles late
            pending_finals = deque()
            slot_idx = 0
            for h in range(NQH):
                yps = yps_pool.tile([P, CH], F32, name=f"yps_{h}_{j}", tag="yps")
                chains = [None] * 4
                ets = [None] * nkb
                qoffs = [0] * nkb

                def emit_av(kb, yps=None, ets=None, qoffs=None):
                    nc.tensor.matmul(
                        yps[:, qoffs[kb] :],
                        lhsT=Vn_sb[:, kb, :],
                        rhs=ets[kb][:, qoffs[kb] :],
                        start=(kb == 0),
                        stop=(kb == nkb - 1),
                    )

                for kb in range(nkb):
                    weave(slot_idx)
                    slot_idx += 1
                    while len(pending_av) > 3:
                        pending_av.popleft()[1]()
                    if pending_finals and kb == 1:
                        pending_finals.popleft()()
                    if kb == 3 and pending_denom[0] is not None:
                        pending_denom[0]()
                        pending_denom[0] = None
                    oi = kb - 4 * j
                    qoff = oi * P if oi > 0 else 0
                    qoffs[kb] = qoff
                    rank2 = _use_rank2(j, h, oi)
                    pss = sc_pool.tile([P, CH], F32, name=f"pss_{h}_{j}_{kb}", tag="sc")
                    nc.tensor.matmul(
                        pss[:, qoff:],
                        lhsT=KT_sb[:, kb * P : (kb + 1) * P],
                        rhs=QT_sb[:, h, q0 + qoff : q0 + CH],
                        start=True,
                        stop=not rank2,
                    )
                    oidx = oi + 12
                    if rank2:
                        nc.tensor.matmul(
                            pss[:, qoff:],
                            lhsT=r2l_sb[:, h, :],
                            rhs=r2r_sb[:, h, qoff:],
                            start=False,
                            stop=True,
                        )
                        bidx = 16 + oidx
                    else:
                        if oi >= 0:
                            nc.vector.tensor_add(
                                pss[:, qoff:], pss[:, qoff:], mcb_sb[:, h, oi, qoff:]
                            )
                        else:
                            nc.vector.tensor_add(pss[:], pss[:], mcl_sb[:, h, :])
                        bidx = oidx
                    et = et_pool.tile([P, CH], F16, name=f"et_{h}_{j}_{kb}", tag="et")
                    ets[kb] = et
                    nc.scalar.activation(
                        et[:, qoff:],
                        pss[:, qoff:],
                        EXP,
                        bias=bias_sb[:, h, bidx : bidx + 1],
                        scale=SCALE,
                    )
                    # AV lags 2 tiles (across head boundaries) so the PE
                    # never waits on the DVE->ACT softmax chain
                    pending_av.append((
                        h,
                        lambda kb=kb, yps=yps, ets=ets, qoffs=qoffs: emit_av(
                            kb, yps, ets, qoffs
                        ),
                    ))
                    # denominator accumulation (4 chains vs rounding)
                    if j == 0:
                        if kb == 0:
                            acc0 = et_pool.tile([P, CH], F16, name=f"ac_{h}", tag="et")
                            nc.vector.tensor_copy(acc0[:], et[:])
                            chains[0] = acc0
                        else:
                            nc.vector.tensor_add(
                                chains[0][:, qoff:], chains[0][:, qoff:], et[:, qoff:]
                            )
                    else:
                        c = (nkb - 1 - kb) % 4
                        if chains[c] is None:
                            chains[c] = et  # head tile of chain: full width
                        else:
                            nc.vector.tensor_add(
                                chains[c][:, qoff:], chains[c][:, qoff:], et[:, qoff:]
                            )
                def emit_finals(chains=chains):
                    if j > 0:
                        nc.vector.tensor_add(chains[0][:], chains[0][:], chains[1][:])
                        nc.vector.tensor_add(chains[2][:], chains[2][:], chains[3][:])
                        nc.vector.tensor_add(chains[0][:], chains[0][:], chains[2][:])

                pending_finals.append(emit_finals)
                pending_denom[0] = (
                    lambda h=h, chains=chains, yps=yps: denom_cluster(
                        h, chains[0], yps
                    )
                )
            while pending_av:
                pending_av.popleft()[1]()
            while pending_finals:
                pending_finals.popleft()()
            if pending_denom[0] is not None:
                pending_denom[0]()
                pending_denom[0] = None
            drain_all_filler()

        # ---------- schedule ----------
        # interleave qwT-quad and xT(0)-quad loads so the first Q matmul
        # can start after ~2 DMAs instead of after 4MB of transfers
        xts_cur = []
        for q in range(4):
            nc.sync.dma_start(qwT_sb[:, 4 * q : 4 * q + 4, :], qwT_ap[:, 4 * q : 4 * q + 4, :])
            xq = xT_pool.tile([P, 4, CH], F16, name=f"xq_0_{q}", tag="xq")
            nc.sync.dma_start(xq[:], xT_ap[:, 4 * q : 4 * q + 4, 0:CH])
            for i in range(4):
                xts_cur.append(xq[:, i, :])
        nc.sync.dma_start(kwT_sb[:], kwT_ap[:])
        nc.sync.dma_start(vwT_sb[:], vwT_ap[:])
        nc.sync.dma_start(id_sb[:], id_ap[:])
        # chunk-0 projections emitted directly (att(0) depends on them)
        unit_qproj(0, xts_cur, 0)()
        # constants needed when attention(0) starts (~20us in)
        nc.sync.dma_start(bias_sb[:], bias_ap[:])
        nc.sync.dma_start(r2l_sb[:], r2l_ap[:])
        nc.sync.dma_start(r2r_sb[:], r2r_ap[:])
        nc.sync.dma_start(ones_sb[:], ones_ap[:])
        nc.sync.dma_start(onesr_sb[:], onesr_ap[:])
        for hh in range(NQH):
            nc.sync.dma_start(mcb_sb[:, hh, :, :], mcb_ap[:, hh, :, :])
        unit_kproj(0, xts_cur)()
        unit_vproj(0, xts_cur)()
        unit_qproj(0, xts_cur, 1)()
        unit_qproj(0, xts_cur, 2)()
        unit_qproj(0, xts_cur, 3)()

        for j in range(NCHK):
            if j < NCHK - 1:
                xts_next = dma_x_chunk(j + 1)
                filler.append(unit_qproj(j + 1, xts_next, 0))
                filler.append(unit_kproj(j + 1, xts_next))
                filler.append(unit_vproj(j + 1, xts_next))
                filler.append(unit_qproj(j + 1, xts_next, 1))
                filler.append(unit_qproj(j + 1, xts_next, 2))
                filler.append(unit_qproj(j + 1, xts_next, 3))
            if j == 0:
                # needed from attention(1) / oproj(0) onward
                nc.sync.dma_start(mcl_sb[:], mcl_ap[:])
                nc.sync.dma_start(owT_sb[:], owT_ap[:])
            # o-projection units woven to even out per-chunk PE load:
            # chunk1 <- oproj(0) x16; chunk2 <- oproj(1) x12;
            # chunk3 <- oproj(1) x4 + oproj(2) x16
            if j == 1:
                for tb in range(0, 4):
                    for nch in range(C // CH):
                        filler.append(unit_oproj(tb, nch))
            elif j == 2:
                for tb in range(4, 7):
                    for nch in range(C // CH):
                        filler.append(unit_oproj(tb, nch))
            elif j == 3:
                for nch in range(C // CH):
                    filler.append(unit_oproj(7, nch))
                for tb in range(8, 12):
                    for nch in range(C // CH):
                        filler.append(unit_oproj(tb, nch))
            attention_chunk(j)

        # tail: o-projection of chunk 3
        for tb in range(12, 16):
            for nch in range(C // CH):
                unit_oproj(tb, nch)()

    nc.compile()
    return nc


def make_in_maps(x, q_w, k_w, v_w, o_w):
    """Host-side sharding/preprocessing -> per-core input dicts."""
    slopes = _alibi_slopes(H)
    x_bf = np.asarray(x, dtype=NP_F16)

    ident = np.eye(P, dtype=NP_F16)

    pi = np.arange(P, dtype=np.float32)[:, None]
    mj = np.arange(CH, dtype=np.float32)[None, :]

    in_maps = []
    for c in range(8):
        b, g = c // 4, c % 4
        qsl = slice(4 * g * P, (4 * g + 4) * P)
        ksl = slice(g * P, (g + 1) * P)

        qwT = np.ascontiguousarray(
            np.asarray(q_w[qsl].T, dtype=NP_F16).reshape(NCB, P, NQH * P).transpose(1, 0, 2)
        )
        kwT = np.ascontiguousarray(
            np.asarray(k_w[ksl].T, dtype=NP_F16).reshape(NCB, P, D).transpose(1, 0, 2)
        )
        vwT = np.ascontiguousarray(
            np.asarray(v_w[ksl].T, dtype=NP_F16).reshape(NCB, P, D).transpose(1, 0, 2)
        )
        owT = np.ascontiguousarray(
            np.asarray(o_w[:, qsl].T, dtype=NP_F16).reshape(NQH, P, C).transpose(1, 0, 2)
        )

        mcl = np.empty((P, NQH, CH), dtype=np.float32)
        mcb = np.empty((P, NQH, 4, CH), dtype=np.float32)
        bias = np.empty((P, NQH, 32), dtype=np.float32)
        r2l = np.empty((2, NQH, P), dtype=NP_F16)
        r2r = np.empty((2, NQH, CH), dtype=NP_F16)
        for h in range(NQH):
            sl = np.float32(slopes[4 * g + h])
            mcl[:, h, :] = (sl / np.float32(SCALE)) * (pi - mj)
            for oi in range(4):
                mcb[:, h, oi, :] = np.where(
                    oi * P + pi - mj > 0.0, np.float32(MASK_NEG), mcl[:, h, :]
                )
            for oidx in range(16):
                bias[:, h, oidx] = sl * np.float32(P * (oidx - 12))
                bias[:, h, 16 + oidx] = sl * np.float32(P * (oidx - 12) - 192.0)
            r2l[0, h, :] = (np.arange(P) - 63.5).astype(NP_F16)
            r2l[1, h, :] = np.float16(1.0)
            r2r[0, h, :] = NP_F16(sl / np.float32(SCALE))
            r2r[1, h, :] = (
                -(sl / np.float32(SCALE)) * (np.arange(CH, dtype=np.float32) - 255.5)
            ).astype(NP_F16)

        xTb = x_bf[b].T  # [C, T]
        in_maps.append(
            dict(
                xT=np.ascontiguousarray(xTb.reshape(NCB, P, T).transpose(1, 0, 2)),
                qwT=qwT,
                kwT=kwT,
                vwT=vwT,
                owT=owT,
                mcl=mcl,
                mcb=mcb,
                bias=bias,
                r2l=r2l,
                r2r=r2r,
                ident=ident,
                ones=np.ones((P, 1), dtype=NP_F16),
                onesr=np.ones((1, P), dtype=NP_F16),
            )
        )
    return in_maps


def gather_output(results):
    out = np.zeros((B, T, C), dtype=np.float32)
    for c in range(8):
        out[c // 4] += results[c]["out_p"].astype(np.float32)
    return out


_NC_CACHE = {}


def get_program():
    if "nc" not in _NC_CACHE:
        _NC_CACHE["nc"] = build_program()
    return _NC_CACHE["nc"]


def kernel(x, q_w, k_w, v_w, o_w):
    from concourse.bass_utils import run_bass_kernel_spmd

    nc = get_program()
    in_maps = make_in_maps(x, q_w, k_w, v_w, o_w)
    res = run_bass_kernel_spmd(nc, in_maps, list(range(8)))
    return gather_output(res.results)
